# revision 7
# baseline (speedup 1.0000x reference)
"""Distributed 2-layer GCN (BangaloreGCN) on 8 Trainium2 NeuronCores.

Strategy (node/graph-parallel, per spec sharding hint):
  * Nodes are packed into 8*49 destination tiles of 128 slots (LPT on
    in-degree so every tile's incoming-edge count fits a fixed chunk
    budget -> fully static SPMD program).
  * GCN algebra is refactored so message passing is a pure gather +
    segment-sum:  out = dinv * (A @ (dinv*h)) + dinv^2 * h, with the
    per-channel BN scale folded into W, biases folded into a post-add.
  * Per layer: each core computes its shard of the (dinv*h) table,
    AllGather replicates it to HBM on every core, then each core
    dma_gathers the source rows for its own edges and segment-sums them
    with one-hot selection matmuls into PSUM (128 dests x 64 feats).
  * int16 gather indices only span 32768 rows, so edges are split into
    a "low" pass (table rows [0, 32768)) and "high" pass (rows
    [NSLOT-32768, NSLOT)); edges in the overlap are assigned to balance
    per-tile chunk counts.
"""

import sys

sys.path.insert(0, "/opt/trn_rl_repo")

import heapq

import ml_dtypes
import numpy as np

BF16 = ml_dtypes.bfloat16

# ---- problem constants (hardcoded per contest contract) ----
N_NODES = 50000
IN_CH = 128
HID = 64
HID2 = 32
BN_EPS = 1e-5

NCORES = 8
P = 128
TILES = 49                 # dest tiles per core
SPC = TILES * P            # slots per core (6272)
NSLOT = NCORES * SPC       # 50176
NBINS = NCORES * TILES
LO_LIM = 32768             # low gather table covers rows [0, 32768)
HI_BASE = NSLOT - 32768    # high table covers [HI_BASE, NSLOT)
GT = 7                     # dest tiles per dma_gather call
NCALLS = TILES // GT
PAD_DEST = 200.0
TBW = 128                  # padded table row width (bf16 -> 256B elems)

USE_BF16 = True


# ----------------------------------------------------------------------
# host-side preparation
# ----------------------------------------------------------------------
def _pack_nodes(deg_in, n):
    order = np.argsort(-deg_in, kind="stable")
    heap = [(0, b) for b in range(NBINS)]
    heapq.heapify(heap)
    counts = np.zeros(NBINS, np.int32)
    binof = np.empty(n, np.int32)
    for v in order:
        load, b = heapq.heappop(heap)
        binof[v] = b
        counts[b] += 1
        if counts[b] < P:
            heapq.heappush(heap, (load + int(deg_in[v]), b))
    perm = np.argsort(binof, kind="stable")
    ptr = np.zeros(NBINS, np.int32)
    lanes = np.empty(n, np.int32)
    for v in perm:
        b = binof[v]
        lanes[v] = ptr[b]
        ptr[b] += 1
    return binof.astype(np.int64) * P + lanes


def _wrap_idx(arr):
    ni = arr.shape[0]
    blk = arr.reshape(ni // 16, 16).T.astype(np.int16)
    return np.tile(blk, (8, 1))


def host_prep(x, edge_index, W1, b1, W2, b2, fcW, fcb,
              g1, be1, rm1, rv1, g2, be2, rm2, rv2):
    n = x.shape[0]
    row = np.asarray(edge_index[0], np.int64)
    col = np.asarray(edge_index[1], np.int64)

    deg = np.bincount(col, minlength=n).astype(np.float32) + 1.0
    dinv = (1.0 / np.sqrt(deg)).astype(np.float32)
    deg_in = np.bincount(col, minlength=n)

    slot_of_node = _pack_nodes(deg_in, n)
    node_of_slot = np.full(NSLOT, -1, np.int64)
    node_of_slot[slot_of_node] = np.arange(n)

    src_slot = slot_of_node[row]
    dst_slot = slot_of_node[col]
    dbin = dst_slot // P
    dlane = dst_slot % P

    order = np.argsort(dbin, kind="stable")
    src_s = src_slot[order]
    dlane_s = dlane[order]
    dbin_s = dbin[order]
    starts = np.searchsorted(dbin_s, np.arange(NBINS))
    ends = np.searchsorted(dbin_s, np.arange(NBINS) + 1)

    nA_min = np.zeros(NBINS, np.int64)
    nB_min = np.zeros(NBINS, np.int64)
    tot = ends - starts
    for b in range(NBINS):
        s = src_s[starts[b]:ends[b]]
        nA_min[b] = int((s < HI_BASE).sum())
        nB_min[b] = int((s >= LO_LIM).sum())
    maxA, maxB, maxT = int(nA_min.max()), int(nB_min.max()), int(tot.max())
    best = None
    for ct in range(-(-maxT // P), -(-maxT // P) + 8):
        for ca in range(-(-maxA // P), ct + 1):
            cb = ct - ca
            if cb >= 0 and cb * P >= maxB:
                best = (ca, cb)
                break
        if best:
            break
    CA, CB = best
    capA, capB = CA * P, CB * P

    srcA = np.zeros((NBINS, capA), np.int64)
    destA = np.full((NBINS, capA), PAD_DEST, np.float32)
    srcB = np.zeros((NBINS, capB), np.int64)
    destB = np.full((NBINS, capB), PAD_DEST, np.float32)
    for b in range(NBINS):
        s = src_s[starts[b]:ends[b]]
        d = dlane_s[starts[b]:ends[b]]
        isB_must = s >= LO_LIM
        isA_must = s < HI_BASE
        mid_idx = np.where(~isB_must & ~isA_must)[0]
        room = capB - int(isB_must.sum())
        takeB = mid_idx[:room]
        selB = np.concatenate([np.where(isB_must)[0], takeB])
        selA = np.concatenate([np.where(isA_must)[0], mid_idx[room:]])
        assert len(selB) <= capB and len(selA) <= capA
        srcB[b, :len(selB)] = s[selB] - HI_BASE
        destB[b, :len(selB)] = d[selB]
        srcA[b, :len(selA)] = s[selA]
        destA[b, :len(selA)] = d[selA]

    S1c = (g1 / np.sqrt(rv1 + BN_EPS)).astype(np.float32)
    T1 = ((b1 - rm1) * S1c + be1).astype(np.float32)
    S2c = (g2 / np.sqrt(rv2 + BN_EPS)).astype(np.float32)
    T2 = ((b2 - rm2) * S2c + be2).astype(np.float32)
    W1p = (W1 * S1c[None, :]).astype(np.float32)
    W2p = (W2 * S2c[None, :]).astype(np.float32)

    NCH = CA + CB
    cores = []
    for c in range(NCORES):
        tsl = slice(c * TILES, (c + 1) * TILES)
        sA = srcA[tsl].reshape(-1)
        sB = srcB[tsl].reshape(-1)
        idxA_img = np.hstack(
            [_wrap_idx(sA[g * GT * capA:(g + 1) * GT * capA]) for g in range(NCALLS)])
        idxB_img = np.hstack(
            [_wrap_idx(sB[g * GT * capB:(g + 1) * GT * capB]) for g in range(NCALLS)])
        dst_img = np.zeros((P, TILES * NCH), np.float32)
        for tl in range(TILES):
            b = c * TILES + tl
            dst_img[:, tl * NCH:tl * NCH + CA] = destA[b].reshape(CA, P).T
            dst_img[:, tl * NCH + CA:(tl + 1) * NCH] = destB[b].reshape(CB, P).T
        nodes = node_of_slot[c * SPC:(c + 1) * SPC]
        occ = nodes >= 0
        xs = np.zeros((SPC, IN_CH), np.float32)
        xs[occ] = x[nodes[occ]] * dinv[nodes[occ], None]
        dv = np.zeros(SPC, np.float32)
        dv[occ] = dinv[nodes[occ]]
        cores.append(dict(
            idxA=idxA_img, idxB=idxB_img,
            dest=dst_img.astype(BF16) if USE_BF16 else dst_img,
            xT=np.ascontiguousarray(xs.T),
            dinv=np.ascontiguousarray(dv.reshape(TILES, P).T),
        ))

    consts = dict(W1p=W1p, W2p=W2p, T1=T1, T2=T2,
                  fcW=np.asarray(fcW, np.float32), fcb=float(np.asarray(fcb).reshape(-1)[0]),
                  CA=CA, CB=CB, node_of_slot=node_of_slot)
    return cores, consts


# ----------------------------------------------------------------------
# device program
# ----------------------------------------------------------------------
def _dma_gather_raw(gp, bassmod, out_ap, in_ap, idxs_ap, num_idxs, elem_size,
                    elem_step, single_packet=True, queue_num=0):
    """bass.dma_gather with elem_size_bytes below 256B allowed (stride must
    still be a multiple of 256B). Verified on HW (see work/smoke4.py)."""
    import concourse.mybir as mybir
    from concourse import ap_utils
    from concourse.bass import MemorySpace, exact_div, round_up_to_multiple

    assert idxs_ap.dtype == mybir.dt.int16
    assert in_ap.dtype == out_ap.dtype
    assert in_ap.space == MemorySpace.DRAM
    assert idxs_ap.space == MemorySpace.SBUF and out_ap.space == MemorySpace.SBUF
    assert ap_utils.ap_is_contiguous(out_ap.ap[1:])
    assert ap_utils.ap_is_contiguous(idxs_ap.ap[1:])
    assert in_ap.ap[-1][1] == out_ap.ap[-1][1] == elem_size
    assert out_ap.ap[0][1] * out_ap.ap[1][1] == round_up_to_multiple(num_idxs, 128)
    assert in_ap.ap[0][0] == elem_step
    stride_bytes_256 = exact_div(elem_step * mybir.dt.size(in_ap.dtype), 256)
    assert stride_bytes_256 < 256
    return gp.add_instruction(
        mybir.InstDMAGatherAnt(
            name=bassmod.get_next_instruction_name(),
            ins=[*gp.lower_ap_dma(in_ap, for_custom_bir_dma=True),
                 gp.lower_ap(idxs_ap),
                 gp.lower_val_access(gp.to_reg(num_idxs))],
            outs=[gp.lower_ap(out_ap)],
            transpose=False,
            num_idxs=num_idxs,
            elem_size=elem_size,
            stride_bytes_256=stride_bytes_256,
            gen_mode=0,
            single_packet=single_packet,
            queue_num=queue_num,
            sbuf_tokens_per_rank=0,
            sbuf_free_dim_per_rank=0,
            sbuf_free_dim_pad_per_rank=0,
            sbuf_byte_offset=0,
        ))


def build_bass(CA, CB):
    import concourse.bacc as bacc
    import concourse.bass as bassm
    import concourse.mybir as mybir
    import concourse.tile as tile
    from concourse.library_config import mlp
    from concourse.masks import make_identity

    f32 = mybir.dt.float32
    bf = mybir.dt.bfloat16 if USE_BF16 else f32
    i16 = mybir.dt.int16
    tbw = TBW if USE_BF16 else HID
    NCH = CA + CB
    capA, capB = CA * P, CB * P
    wA = GT * capA // 16
    wB = GT * capB // 16

    nc = bacc.Bacc("TRN2", target_bir_lowering=False)
    xT_d = nc.dram_tensor("xT", [P, SPC], f32, kind="ExternalInput")
    idxA_d = nc.dram_tensor("idxA", [P, TILES * capA // 16], i16, kind="ExternalInput")
    idxB_d = nc.dram_tensor("idxB", [P, TILES * capB // 16], i16, kind="ExternalInput")
    dest_d = nc.dram_tensor("dest", [P, TILES * NCH], bf, kind="ExternalInput")
    dinv_d = nc.dram_tensor("dinv", [P, TILES], f32, kind="ExternalInput")
    w1_d = nc.dram_tensor("w1", [IN_CH, HID], f32, kind="ExternalInput")
    w2_d = nc.dram_tensor("w2", [HID, HID2], bf, kind="ExternalInput")
    t1_d = nc.dram_tensor("t1", [P, HID], f32, kind="ExternalInput")
    t2_d = nc.dram_tensor("t2", [P, HID2], f32, kind="ExternalInput")
    fcw_d = nc.dram_tensor("fcw", [P, HID2], f32, kind="ExternalInput")
    y_d = nc.dram_tensor("y", [P, TILES], f32, kind="ExternalOutput")

    with tile.TileContext(nc) as tc:
        with (
            tc.tile_pool(name="const", bufs=1) as cpool,
            tc.tile_pool(name="upart", bufs=1) as upool,
            tc.tile_pool(name="xtile", bufs=3) as xpool,
            tc.tile_pool(name="ga", bufs=2) as gapool,
            tc.tile_pool(name="gb", bufs=2) as gbpool,
            tc.tile_pool(name="sel", bufs=3) as selpool,
            tc.tile_pool(name="work", bufs=4) as wpool,
            tc.tile_pool(name="pmm", bufs=2, space="PSUM") as pmm,
            tc.tile_pool(name="pacc", bufs=3, space="PSUM") as pacc,
            tc.tile_pool(name="ptr", bufs=1, space="PSUM") as ptr,
            tc.tile_pool(name="p3", bufs=2, space="PSUM") as p3pool,
            tc.tile_pool(name="dram", bufs=1, space="DRAM") as dpool,
        ):
            nc.gpsimd.load_library(mlp)

            # ---- constants ----
            idxA_t = cpool.tile([P, TILES * capA // 16], i16)
            nc.sync.dma_start(out=idxA_t[:], in_=idxA_d[:])
            idxB_t = cpool.tile([P, TILES * capB // 16], i16)
            nc.sync.dma_start(out=idxB_t[:], in_=idxB_d[:])
            dest_t = cpool.tile([P, TILES * NCH], bf)
            nc.sync.dma_start(out=dest_t[:], in_=dest_d[:])
            dinv_t = cpool.tile([P, TILES], f32)
            nc.sync.dma_start(out=dinv_t[:], in_=dinv_d[:])
            w1_t = cpool.tile([IN_CH, HID], f32)
            nc.sync.dma_start(out=w1_t[:], in_=w1_d[:])
            w2_t = cpool.tile([HID, HID2], bf)
            nc.sync.dma_start(out=w2_t[:], in_=w2_d[:])
            t1_t = cpool.tile([P, HID], f32)
            nc.sync.dma_start(out=t1_t[:], in_=t1_d[:])
            t2_t = cpool.tile([P, HID2], f32)
            nc.sync.dma_start(out=t2_t[:], in_=t2_d[:])
            fcw_t = cpool.tile([P, HID2], f32)
            nc.sync.dma_start(out=fcw_t[:], in_=fcw_d[:])

            ident = cpool.tile([P, P], f32)
            make_identity(nc, ident[:])
            iota_i = cpool.tile([P, NCH * P], mybir.dt.int32)
            nc.gpsimd.iota(iota_i[:], pattern=[[0, NCH], [1, P]], base=0,
                           channel_multiplier=0)
            iota_b = cpool.tile([P, NCH * P], bf)
            nc.vector.tensor_copy(out=iota_b[:], in_=iota_i[:])

            u1_t = upool.tile([P, TILES * HID], f32, tag="u1")
            s2_t = upool.tile([P, TILES * HID], f32, tag="s2")
            out_t = upool.tile([P, TILES], f32, tag="out")

            ag1_in = dpool.tile([SPC, tbw], bf)
            s1_tab = dpool.tile([NSLOT, tbw], bf, addr_space="Shared")
            ag2_in = dpool.tile([SPC, tbw], bf)
            s2_tab = dpool.tile([NSLOT, tbw], bf, addr_space="Shared")

            # ---- L1 dense: u = (x*dinv) @ W1' ----
            for t in range(TILES):
                xt = xpool.tile([P, P], f32, tag="xt")
                nc.sync.dma_start(out=xt[:], in_=xT_d[:, t * P:(t + 1) * P])
                pm = pmm.tile([P, HID], f32, space="PSUM", tag="pm")
                nc.tensor.matmul(out=pm[:], lhsT=xt[:], rhs=w1_t[:],
                                 start=True, stop=True)
                nc.vector.tensor_copy(out=u1_t[:, t * HID:(t + 1) * HID], in_=pm[:])

            nc.gpsimd.dma_start(
                out=ag1_in[:].rearrange("(t p) w -> p t w", p=P)[:, :, 0:HID],
                in_=u1_t[:].rearrange("p (t f) -> p t f", f=HID),
            )
            nc.gpsimd.collective_compute(
                "AllGather", mybir.AluOpType.bypass,
                replica_groups=[list(range(NCORES))],
                ins=[ag1_in[:]], outs=[s1_tab[:]],
            )

            def tab_ap(tab, lo, cnt):
                return bassm.AP(tensor=tab[:].tensor, offset=lo * tbw,
                                ap=[[tbw, cnt], [1, HID]])

            def scatter_tiles(tab, post):
                for g in range(NCALLS):
                    ga = gapool.tile([P, GT * CA, HID], bf, tag="ga")
                    _dma_gather_raw(
                        nc.gpsimd, nc, ga[:], tab_ap(tab, 0, LO_LIM),
                        idxA_t[:, g * wA:(g + 1) * wA], GT * capA, HID, tbw,
                        single_packet=False)
                    gb = gbpool.tile([P, GT * CB, HID], bf, tag="gb")
                    _dma_gather_raw(
                        nc.gpsimd, nc, gb[:], tab_ap(tab, HI_BASE, LO_LIM),
                        idxB_t[:, g * wB:(g + 1) * wB], GT * capB, HID, tbw,
                        single_packet=False)
                    for k in range(GT):
                        t = g * GT + k
                        sel = selpool.tile([P, NCH, P], bf, tag="sel")
                        nc.vector.tensor_tensor(
                            out=sel[:],
                            in0=dest_t[:, t * NCH:(t + 1) * NCH, None]
                                .to_broadcast([P, NCH, P]),
                            in1=iota_b[:].rearrange("p (c q) -> p c q", q=P),
                            op=mybir.AluOpType.is_equal,
                        )
                        acc = pacc.tile([P, HID], f32, space="PSUM", tag="acc")
                        for cc in range(NCH):
                            rhs = (ga[:, k * CA + cc, :] if cc < CA
                                   else gb[:, k * CB + cc - CA, :])
                            nc.tensor.matmul(out=acc[:], lhsT=sel[:, cc, :], rhs=rhs,
                                             start=(cc == 0), stop=(cc == NCH - 1))
                        post(t, acc)

            # ---- L1 scatter + post: s2 = dinv * relu(dinv*(acc+u) + T1) ----
            def post1(t, acc):
                tmp = wpool.tile([P, HID], f32, tag="tmp1")
                nc.vector.tensor_tensor(out=tmp[:], in0=acc[:],
                                        in1=u1_t[:, t * HID:(t + 1) * HID],
                                        op=mybir.AluOpType.add)
                nc.vector.tensor_tensor(out=tmp[:], in0=tmp[:],
                                        in1=dinv_t[:, t:t + 1].to_broadcast([P, HID]),
                                        op=mybir.AluOpType.mult)
                nc.vector.tensor_tensor(out=tmp[:], in0=tmp[:], in1=t1_t[:],
                                        op=mybir.AluOpType.add)
                h1 = wpool.tile([P, HID], f32, tag="h1")
                nc.scalar.activation(out=h1[:], in_=tmp[:],
                                     func=mybir.ActivationFunctionType.Relu)
                nc.vector.tensor_tensor(out=s2_t[:, t * HID:(t + 1) * HID],
                                        in0=h1[:],
                                        in1=dinv_t[:, t:t + 1].to_broadcast([P, HID]),
                                        op=mybir.AluOpType.mult)

            scatter_tiles(s1_tab, post1)

            nc.gpsimd.dma_start(
                out=ag2_in[:].rearrange("(t p) w -> p t w", p=P)[:, :, 0:HID],
                in_=s2_t[:].rearrange("p (t f) -> p t f", f=HID),
            )
            nc.gpsimd.collective_compute(
                "AllGather", mybir.AluOpType.bypass,
                replica_groups=[list(range(NCORES))],
                ins=[ag2_in[:]], outs=[s2_tab[:]],
            )

            # ---- L2 scatter + post ----
            def post2(t, acc):
                agg = wpool.tile([P, HID], f32, tag="agg")
                nc.vector.tensor_tensor(out=agg[:], in0=acc[:],
                                        in1=s2_t[:, t * HID:(t + 1) * HID],
                                        op=mybir.AluOpType.add)
                nc.vector.tensor_tensor(out=agg[:], in0=agg[:],
                                        in1=dinv_t[:, t:t + 1].to_broadcast([P, HID]),
                                        op=mybir.AluOpType.mult)
                trp = ptr.tile([HID, P], f32, space="PSUM", tag="trp")
                nc.tensor.transpose(out=trp[:], in_=agg[:], identity=ident[:])
                aggT = wpool.tile([HID, P], bf, tag="aggT")
                nc.vector.tensor_copy(out=aggT[:], in_=trp[:])
                p3 = p3pool.tile([P, HID2], f32, space="PSUM", tag="p3")
                nc.tensor.matmul(out=p3[:], lhsT=aggT[:], rhs=w2_t[:],
                                 start=True, stop=True)
                h2p = wpool.tile([P, HID2], f32, tag="h2p")
                nc.vector.tensor_tensor(out=h2p[:], in0=p3[:], in1=t2_t[:],
                                        op=mybir.AluOpType.add)
                h2 = wpool.tile([P, HID2], f32, tag="h2")
                nc.scalar.activation(out=h2[:], in_=h2p[:],
                                     func=mybir.ActivationFunctionType.Relu)
                prod = wpool.tile([P, HID2], f32, tag="prod")
                nc.vector.tensor_tensor(out=prod[:], in0=h2[:], in1=fcw_t[:],
                                        op=mybir.AluOpType.mult)
                nc.vector.reduce_sum(out=out_t[:, t:t + 1], in_=prod[:],
                                     axis=mybir.AxisListType.X)

            scatter_tiles(s2_tab, post2)

            nc.sync.dma_start(out=y_d[:], in_=out_t[:])

    nc.compile()
    return nc


# ----------------------------------------------------------------------
# entry point
# ----------------------------------------------------------------------
def prepare(inputs):
    inputs = {k: np.asarray(v) for k, v in inputs.items()}
    cores, consts = host_prep(**inputs)
    nc = build_bass(consts["CA"], consts["CB"])

    cast = BF16 if USE_BF16 else np.float32
    w2 = consts["W2p"].astype(cast)
    t1 = np.tile(consts["T1"][None, :], (P, 1)).astype(np.float32)
    t2 = np.tile(consts["T2"][None, :], (P, 1)).astype(np.float32)
    fcw = np.tile(consts["fcW"].reshape(1, -1), (P, 1)).astype(np.float32)

    in_maps = []
    for c in range(NCORES):
        in_maps.append({
            "xT": cores[c]["xT"],
            "idxA": cores[c]["idxA"],
            "idxB": cores[c]["idxB"],
            "dest": cores[c]["dest"].astype(cast),
            "dinv": cores[c]["dinv"],
            "w1": consts["W1p"],
            "w2": w2,
            "t1": t1,
            "t2": t2,
            "fcw": fcw,
        })
    return nc, in_maps, consts


def execute(nc, in_maps):
    from concourse.bass_utils import run_bass_kernel_spmd
    return run_bass_kernel_spmd(nc, in_maps, core_ids=list(range(NCORES)))


def unshard(res, consts):
    y = np.zeros((N_NODES, 1), np.float32)
    nos = consts["node_of_slot"]
    fcb = consts["fcb"]
    for c in range(NCORES):
        nodes = nos[c * SPC:(c + 1) * SPC]
        occ = nodes >= 0
        vals = res.results[c]["y"].T.reshape(-1) + fcb
        y[nodes[occ], 0] = vals[occ]
    return y


def kernel(**inputs):
    nc, in_maps, consts = prepare(inputs)
    res = execute(nc, in_maps)
    return unshard(res, consts)


# revision 17
# speedup vs baseline: 1740.5185x; 1740.5185x over previous
"""Distributed 2-layer GCN (BangaloreGCN) on 8 Trainium2 NeuronCores.

Strategy (node/graph-parallel, per spec sharding hint):
  * Nodes are packed into 8*49 destination tiles of 128 slots (LPT on
    in-degree so every tile's incoming-edge count fits a fixed chunk
    budget -> fully static SPMD program).
  * GCN algebra is refactored so message passing is a pure gather +
    segment-sum:  out = dinv * (A @ (dinv*h)) + dinv^2 * h, with the
    per-channel BN scale folded into W, biases folded into a post-add.
  * Per layer: each core computes its shard of the (dinv*h) table,
    AllGather replicates it to HBM on every core, then each core
    dma_gathers the source rows for its own edges and segment-sums them
    with one-hot selection matmuls into PSUM (128 dests x 64 feats).
  * int16 gather indices only span 32768 rows, so edges are split into
    a "low" pass (table rows [0, 32768)) and "high" pass (rows
    [NSLOT-32768, NSLOT)); edges in the overlap are assigned to balance
    per-tile chunk counts.
"""

import sys

sys.path.insert(0, "/opt/trn_rl_repo")

import heapq

import ml_dtypes
import numpy as np

BF16 = ml_dtypes.bfloat16

# ---- problem constants (hardcoded per contest contract) ----
N_NODES = 50000
IN_CH = 128
HID = 64
HID2 = 32
BN_EPS = 1e-5

NCORES = 8
P = 128
TILES = 49                 # dest tiles per core
SPC = TILES * P            # slots per core (6272)
NSLOT = NCORES * SPC       # 50176
NBINS = NCORES * TILES
LO_LIM = 32768             # low gather table covers rows [0, 32768)
HI_BASE = NSLOT - 32768    # high table covers [HI_BASE, NSLOT)
GT = 7                     # dest tiles per dma_gather call
NCALLS = TILES // GT
PAD_DEST = 200.0
TBW = 128                  # padded table row width (bf16 -> 256B elems)

USE_BF16 = True


# ----------------------------------------------------------------------
# host-side preparation
# ----------------------------------------------------------------------
def _pack_nodes(deg_in, n):
    order = np.argsort(-deg_in, kind="stable")
    heap = [(0, b) for b in range(NBINS)]
    heapq.heapify(heap)
    counts = np.zeros(NBINS, np.int32)
    binof = np.empty(n, np.int32)
    for v in order:
        load, b = heapq.heappop(heap)
        binof[v] = b
        counts[b] += 1
        if counts[b] < P:
            heapq.heappush(heap, (load + int(deg_in[v]), b))
    perm = np.argsort(binof, kind="stable")
    ptr = np.zeros(NBINS, np.int32)
    lanes = np.empty(n, np.int32)
    for v in perm:
        b = binof[v]
        lanes[v] = ptr[b]
        ptr[b] += 1
    return binof.astype(np.int64) * P + lanes


def _wrap_idx(arr):
    ni = arr.shape[0]
    blk = arr.reshape(ni // 16, 16).T.astype(np.int16)
    return np.tile(blk, (8, 1))


def host_prep(x, edge_index, W1, b1, W2, b2, fcW, fcb,
              g1, be1, rm1, rv1, g2, be2, rm2, rv2):
    n = x.shape[0]
    row = np.asarray(edge_index[0], np.int64)
    col = np.asarray(edge_index[1], np.int64)

    deg = np.bincount(col, minlength=n).astype(np.float32) + 1.0
    dinv = (1.0 / np.sqrt(deg)).astype(np.float32)
    deg_in = np.bincount(col, minlength=n)

    slot_of_node = _pack_nodes(deg_in, n)
    node_of_slot = np.full(NSLOT, -1, np.int64)
    node_of_slot[slot_of_node] = np.arange(n)

    src_slot = slot_of_node[row]
    dst_slot = slot_of_node[col]
    dbin = dst_slot // P
    dlane = dst_slot % P

    order = np.argsort(dbin, kind="stable")
    src_s = src_slot[order]
    dlane_s = dlane[order]
    dbin_s = dbin[order]
    starts = np.searchsorted(dbin_s, np.arange(NBINS))
    ends = np.searchsorted(dbin_s, np.arange(NBINS) + 1)

    nA_min = np.zeros(NBINS, np.int64)
    nB_min = np.zeros(NBINS, np.int64)
    tot = ends - starts
    for b in range(NBINS):
        s = src_s[starts[b]:ends[b]]
        nA_min[b] = int((s < HI_BASE).sum())
        nB_min[b] = int((s >= LO_LIM).sum())
    maxA, maxB, maxT = int(nA_min.max()), int(nB_min.max()), int(tot.max())
    best = None
    for ct in range(-(-maxT // P), -(-maxT // P) + 8):
        for ca in range(-(-maxA // P), ct + 1):
            cb = ct - ca
            if cb >= 0 and cb * P >= maxB:
                best = (ca, cb)
                break
        if best:
            break
    CA, CB = best
    capA, capB = CA * P, CB * P

    srcA = np.zeros((NBINS, capA), np.int64)
    destA = np.full((NBINS, capA), PAD_DEST, np.float32)
    srcB = np.zeros((NBINS, capB), np.int64)
    destB = np.full((NBINS, capB), PAD_DEST, np.float32)
    for b in range(NBINS):
        s = src_s[starts[b]:ends[b]]
        d = dlane_s[starts[b]:ends[b]]
        isB_must = s >= LO_LIM
        isA_must = s < HI_BASE
        mid_idx = np.where(~isB_must & ~isA_must)[0]
        room = capB - int(isB_must.sum())
        takeB = mid_idx[:room]
        selB = np.concatenate([np.where(isB_must)[0], takeB])
        selA = np.concatenate([np.where(isA_must)[0], mid_idx[room:]])
        assert len(selB) <= capB and len(selA) <= capA
        srcB[b, :len(selB)] = s[selB] - HI_BASE
        destB[b, :len(selB)] = d[selB]
        srcA[b, :len(selA)] = s[selA]
        destA[b, :len(selA)] = d[selA]

    S1c = (g1 / np.sqrt(rv1 + BN_EPS)).astype(np.float32)
    T1 = ((b1 - rm1) * S1c + be1).astype(np.float32)
    S2c = (g2 / np.sqrt(rv2 + BN_EPS)).astype(np.float32)
    T2 = ((b2 - rm2) * S2c + be2).astype(np.float32)
    W1p = (W1 * S1c[None, :]).astype(np.float32)
    W2p = (W2 * S2c[None, :]).astype(np.float32)

    NCH = CA + CB
    cores = []
    for c in range(NCORES):
        tsl = slice(c * TILES, (c + 1) * TILES)
        sA = srcA[tsl].reshape(-1)
        sB = srcB[tsl].reshape(-1)
        idxA_img = np.hstack(
            [_wrap_idx(sA[g * GT * capA:(g + 1) * GT * capA]) for g in range(NCALLS)])
        idxB_img = np.hstack(
            [_wrap_idx(sB[g * GT * capB:(g + 1) * GT * capB]) for g in range(NCALLS)])
        dst_img = np.zeros((P, TILES * NCH), np.float32)
        for tl in range(TILES):
            b = c * TILES + tl
            dst_img[:, tl * NCH:tl * NCH + CA] = destA[b].reshape(CA, P).T
            dst_img[:, tl * NCH + CA:(tl + 1) * NCH] = destB[b].reshape(CB, P).T
        nodes = node_of_slot[c * SPC:(c + 1) * SPC]
        occ = nodes >= 0
        xs = np.zeros((SPC, IN_CH), np.float32)
        xs[occ] = x[nodes[occ]] * dinv[nodes[occ], None]
        dv = np.zeros(SPC, np.float32)
        dv[occ] = dinv[nodes[occ]]
        cores.append(dict(
            idxA=idxA_img, idxB=idxB_img,
            dest=dst_img.astype(BF16) if USE_BF16 else dst_img,
            xT=np.ascontiguousarray(xs.T),
            dinv=np.ascontiguousarray(dv.reshape(TILES, P).T),
        ))

    consts = dict(W1p=W1p, W2p=W2p, T1=T1, T2=T2,
                  fcW=np.asarray(fcW, np.float32), fcb=float(np.asarray(fcb).reshape(-1)[0]),
                  CA=CA, CB=CB, node_of_slot=node_of_slot)
    return cores, consts


# ----------------------------------------------------------------------
# device program
# ----------------------------------------------------------------------
def _dma_gather_raw(gp, bassmod, out_ap, in_ap, idxs_ap, num_idxs, elem_size,
                    elem_step, single_packet=True, queue_num=0):
    """bass.dma_gather with elem_size_bytes below 256B allowed (stride must
    still be a multiple of 256B). Verified on HW (see work/smoke4.py)."""
    import concourse.mybir as mybir
    from concourse import ap_utils
    from concourse.bass import MemorySpace, exact_div, round_up_to_multiple

    assert idxs_ap.dtype == mybir.dt.int16
    assert in_ap.dtype == out_ap.dtype
    assert in_ap.space == MemorySpace.DRAM
    assert idxs_ap.space == MemorySpace.SBUF and out_ap.space == MemorySpace.SBUF
    assert ap_utils.ap_is_contiguous(out_ap.ap[1:])
    assert ap_utils.ap_is_contiguous(idxs_ap.ap[1:])
    assert in_ap.ap[-1][1] == out_ap.ap[-1][1] == elem_size
    assert out_ap.ap[0][1] * out_ap.ap[1][1] == round_up_to_multiple(num_idxs, 128)
    assert in_ap.ap[0][0] == elem_step
    stride_bytes_256 = exact_div(elem_step * mybir.dt.size(in_ap.dtype), 256)
    assert stride_bytes_256 < 256
    return gp.add_instruction(
        mybir.InstDMAGatherAnt(
            name=bassmod.get_next_instruction_name(),
            ins=[*gp.lower_ap_dma(in_ap, for_custom_bir_dma=True),
                 gp.lower_ap(idxs_ap),
                 gp.lower_val_access(gp.to_reg(num_idxs))],
            outs=[gp.lower_ap(out_ap)],
            transpose=False,
            num_idxs=num_idxs,
            elem_size=elem_size,
            stride_bytes_256=stride_bytes_256,
            gen_mode=0,
            single_packet=single_packet,
            queue_num=queue_num,
            sbuf_tokens_per_rank=0,
            sbuf_free_dim_per_rank=0,
            sbuf_free_dim_pad_per_rank=0,
            sbuf_byte_offset=0,
        ))


def build_bass(CA, CB):
    import concourse.bacc as bacc
    import concourse.bass as bassm
    import concourse.mybir as mybir
    import concourse.tile as tile
    from concourse.library_config import mlp
    from concourse.masks import make_identity

    f32 = mybir.dt.float32
    bf = mybir.dt.bfloat16 if USE_BF16 else f32
    i16 = mybir.dt.int16
    tbw = TBW if USE_BF16 else HID
    NCH = CA + CB
    capA, capB = CA * P, CB * P
    wA = GT * capA // 16
    wB = GT * capB // 16

    nc = bacc.Bacc("TRN2", target_bir_lowering=False)
    xT_d = nc.dram_tensor("xT", [P, SPC], bf, kind="ExternalInput")
    idxA_d = nc.dram_tensor("idxA", [P, TILES * capA // 16], i16, kind="ExternalInput")
    idxB_d = nc.dram_tensor("idxB", [P, TILES * capB // 16], i16, kind="ExternalInput")
    dest_d = nc.dram_tensor("dest", [P, TILES * NCH], bf, kind="ExternalInput")
    dinv_d = nc.dram_tensor("dinv", [P, TILES], f32, kind="ExternalInput")
    w1_d = nc.dram_tensor("w1", [IN_CH, HID], bf, kind="ExternalInput")
    w2_d = nc.dram_tensor("w2", [HID, HID2], f32, kind="ExternalInput")
    t1_d = nc.dram_tensor("t1", [P, HID], f32, kind="ExternalInput")
    t2_d = nc.dram_tensor("t2", [P, HID2], f32, kind="ExternalInput")
    fcw_d = nc.dram_tensor("fcw", [P, HID2], f32, kind="ExternalInput")
    y_d = nc.dram_tensor("y", [P, TILES], f32, kind="ExternalOutput")

    with tile.TileContext(nc) as tc:
        with (
            tc.tile_pool(name="const", bufs=1) as cpool,
            tc.tile_pool(name="upart", bufs=1) as upool,
            tc.tile_pool(name="ga", bufs=3) as gapool,
            tc.tile_pool(name="gb", bufs=2) as gbpool,
            tc.tile_pool(name="sel", bufs=20) as selpool,
            tc.tile_pool(name="work", bufs=4) as wpool,
            tc.tile_pool(name="pmm", bufs=2, space="PSUM") as pmm,
            tc.tile_pool(name="pacc", bufs=3, space="PSUM") as pacc,
            tc.tile_pool(name="ptr", bufs=1, space="PSUM") as ptr,
            tc.tile_pool(name="p3", bufs=2, space="PSUM") as p3pool,
            tc.tile_pool(name="dram", bufs=1, space="DRAM") as dpool,
        ):
            nc.gpsimd.load_library(mlp)

            # ---- constants ----
            idxA_t = cpool.tile([P, TILES * capA // 16], i16)
            nc.sync.dma_start(out=idxA_t[:], in_=idxA_d[:])
            idxB_t = cpool.tile([P, TILES * capB // 16], i16)
            nc.sync.dma_start(out=idxB_t[:], in_=idxB_d[:])
            dest_t = cpool.tile([P, TILES * NCH], bf)
            nc.sync.dma_start(out=dest_t[:], in_=dest_d[:])
            dinv_t = cpool.tile([P, TILES], f32)
            nc.sync.dma_start(out=dinv_t[:], in_=dinv_d[:])
            w1_t = cpool.tile([IN_CH, HID], bf)
            nc.sync.dma_start(out=w1_t[:], in_=w1_d[:])
            w2_t = cpool.tile([HID, HID2], f32)
            nc.sync.dma_start(out=w2_t[:], in_=w2_d[:])
            t1_t = cpool.tile([P, HID], f32)
            nc.sync.dma_start(out=t1_t[:], in_=t1_d[:])
            t2_t = cpool.tile([P, HID2], f32)
            nc.sync.dma_start(out=t2_t[:], in_=t2_d[:])
            fcw_t = cpool.tile([P, HID2], f32)
            nc.sync.dma_start(out=fcw_t[:], in_=fcw_d[:])

            ident = cpool.tile([P, P], f32)
            make_identity(nc, ident[:])
            ones_row = cpool.tile([1, P], f32)
            nc.gpsimd.memset(ones_row[:], 1.0)
            iota_i = cpool.tile([P, NCH * P], mybir.dt.int32)
            nc.gpsimd.iota(iota_i[:], pattern=[[0, NCH], [1, P]], base=0,
                           channel_multiplier=0)
            iota_b = cpool.tile([P, NCH * P], bf)
            nc.vector.tensor_copy(out=iota_b[:], in_=iota_i[:])

            u1_t = upool.tile([P, TILES * HID], f32, tag="u1")
            s2_t = upool.tile([P, TILES * HID], f32, tag="s2")
            out_t = upool.tile([P, TILES], f32, tag="out")

            ag1_in = dpool.tile([SPC, tbw], bf)
            s1_tab = dpool.tile([NSLOT, tbw], bf, addr_space="Shared")
            ag2_in = dpool.tile([SPC, tbw], bf)
            s2_tab = dpool.tile([NSLOT, tbw], bf, addr_space="Shared")

            # ---- L1 dense: u = (x*dinv) @ W1' ----
            xfull = cpool.tile([P, SPC], bf)
            nc.sync.dma_start(out=xfull[:], in_=xT_d[:])
            for t in range(TILES):
                pm = pmm.tile([P, HID], f32, space="PSUM", tag="pm")
                nc.tensor.matmul(out=pm[:], lhsT=xfull[:, t * P:(t + 1) * P],
                                 rhs=w1_t[:], start=True, stop=True)
                nc.vector.tensor_copy(out=u1_t[:, t * HID:(t + 1) * HID], in_=pm[:])

            nc.gpsimd.dma_start(
                out=ag1_in[:].rearrange("(t p) w -> p t w", p=P)[:, :, 0:HID],
                in_=u1_t[:].rearrange("p (t f) -> p t f", f=HID),
            )
            nc.gpsimd.collective_compute(
                "AllGather", mybir.AluOpType.bypass,
                replica_groups=[list(range(NCORES))],
                ins=[ag1_in[:]], outs=[s1_tab[:]],
            )

            def tab_ap(tab, lo, cnt):
                return bassm.AP(tensor=tab[:].tensor, offset=lo * tbw,
                                ap=[[tbw, cnt], [1, HID]])

            def scatter_tiles(tab, post):
                for g in range(NCALLS):
                    ga = gapool.tile([P, GT * CA, HID], bf, tag="ga")
                    _dma_gather_raw(
                        nc.gpsimd, nc, ga[:], tab_ap(tab, 0, LO_LIM),
                        idxA_t[:, g * wA:(g + 1) * wA], GT * capA, HID, tbw,
                        single_packet=False)
                    gb = gbpool.tile([P, GT * CB, HID], bf, tag="gb")
                    _dma_gather_raw(
                        nc.gpsimd, nc, gb[:], tab_ap(tab, HI_BASE, LO_LIM),
                        idxB_t[:, g * wB:(g + 1) * wB], GT * capB, HID, tbw,
                        single_packet=False)
                    for k in range(GT):
                        t = g * GT + k
                        sel = selpool.tile([P, NCH, P], bf, tag="sel")
                        nc.vector.tensor_tensor(
                            out=sel[:],
                            in0=dest_t[:, t * NCH:(t + 1) * NCH, None]
                                .to_broadcast([P, NCH, P]),
                            in1=iota_b[:].rearrange("p (c q) -> p c q", q=P),
                            op=mybir.AluOpType.is_equal,
                        )
                        acc = pacc.tile([P, HID], f32, space="PSUM", tag="acc")
                        for cc in range(NCH):
                            rhs = (ga[:, k * CA + cc, :] if cc < CA
                                   else gb[:, k * CB + cc - CA, :])
                            nc.tensor.matmul(out=acc[:], lhsT=sel[:, cc, :], rhs=rhs,
                                             start=(cc == 0), stop=(cc == NCH - 1))
                        post(t, acc)

            # ---- L1 scatter + post: s2 = dinv * relu(dinv*(acc+u) + T1) ----
            def post1(t, acc):
                tmp = wpool.tile([P, HID], f32, tag="tmp1")
                nc.vector.tensor_tensor(out=tmp[:], in0=acc[:],
                                        in1=u1_t[:, t * HID:(t + 1) * HID],
                                        op=mybir.AluOpType.add)
                tmp2 = wpool.tile([P, HID], f32, tag="tmp2")
                nc.scalar.activation(out=tmp2[:], in_=tmp[:],
                                     func=mybir.ActivationFunctionType.Copy,
                                     scale=dinv_t[:, t:t + 1])
                h1 = wpool.tile([P, HID], f32, tag="h1")
                nc.vector.tensor_tensor(out=h1[:], in0=tmp2[:], in1=t1_t[:],
                                        op=mybir.AluOpType.add)
                h1r = wpool.tile([P, HID], f32, tag="h1r")
                nc.scalar.activation(out=h1r[:], in_=h1[:],
                                     func=mybir.ActivationFunctionType.Relu)
                nc.scalar.activation(out=s2_t[:, t * HID:(t + 1) * HID],
                                     in_=h1r[:],
                                     func=mybir.ActivationFunctionType.Copy,
                                     scale=dinv_t[:, t:t + 1])

            scatter_tiles(s1_tab, post1)

            nc.gpsimd.dma_start(
                out=ag2_in[:].rearrange("(t p) w -> p t w", p=P)[:, :, 0:HID],
                in_=s2_t[:].rearrange("p (t f) -> p t f", f=HID),
            )
            nc.gpsimd.collective_compute(
                "AllGather", mybir.AluOpType.bypass,
                replica_groups=[list(range(NCORES))],
                ins=[ag2_in[:]], outs=[s2_tab[:]],
            )

            # ---- L2 scatter + post ----
            def post2(t, acc):
                agg0 = wpool.tile([P, HID], f32, tag="agg0")
                nc.vector.tensor_tensor(out=agg0[:], in0=acc[:],
                                        in1=s2_t[:, t * HID:(t + 1) * HID],
                                        op=mybir.AluOpType.add)
                agg = wpool.tile([P, HID], f32, tag="agg")
                nc.scalar.activation(out=agg[:], in_=agg0[:],
                                     func=mybir.ActivationFunctionType.Copy,
                                     scale=dinv_t[:, t:t + 1])
                trp = ptr.tile([HID, P], f32, space="PSUM", tag="trp")
                nc.tensor.transpose(out=trp[:], in_=agg[:], identity=ident[:])
                aggT = wpool.tile([HID, P], f32, tag="aggT")
                nc.scalar.activation(out=aggT[:], in_=trp[:],
                                     func=mybir.ActivationFunctionType.Copy)
                p3 = p3pool.tile([P, HID2], f32, space="PSUM", tag="p3")
                nc.tensor.matmul(out=p3[:], lhsT=aggT[:], rhs=w2_t[:],
                                 start=True, stop=False)
                nc.tensor.matmul(out=p3[:], lhsT=ones_row[:], rhs=t2_t[0:1, :],
                                 start=False, stop=True)
                h2 = wpool.tile([P, HID2], f32, tag="h2")
                nc.scalar.activation(out=h2[:], in_=p3[:],
                                     func=mybir.ActivationFunctionType.Relu)
                prod = wpool.tile([P, HID2], f32, tag="prod")
                nc.vector.tensor_tensor(out=prod[:], in0=h2[:], in1=fcw_t[:],
                                        op=mybir.AluOpType.mult)
                nc.vector.reduce_sum(out=out_t[:, t:t + 1], in_=prod[:],
                                     axis=mybir.AxisListType.X)

            scatter_tiles(s2_tab, post2)

            nc.sync.dma_start(out=y_d[:], in_=out_t[:])

    nc.compile()
    return nc


# ----------------------------------------------------------------------
# entry point
# ----------------------------------------------------------------------
def prepare(inputs):
    inputs = {k: np.asarray(v) for k, v in inputs.items()}
    cores, consts = host_prep(**inputs)
    nc = build_bass(consts["CA"], consts["CB"])

    cast = BF16 if USE_BF16 else np.float32
    w2 = consts["W2p"].astype(np.float32)
    t1 = np.tile(consts["T1"][None, :], (P, 1)).astype(np.float32)
    t2 = np.tile(consts["T2"][None, :], (P, 1)).astype(np.float32)
    fcw = np.tile(consts["fcW"].reshape(1, -1), (P, 1)).astype(np.float32)

    in_maps = []
    for c in range(NCORES):
        in_maps.append({
            "xT": cores[c]["xT"].astype(BF16) if USE_BF16 else cores[c]["xT"],
            "idxA": cores[c]["idxA"],
            "idxB": cores[c]["idxB"],
            "dest": cores[c]["dest"].astype(cast),
            "dinv": cores[c]["dinv"],
            "w1": consts["W1p"].astype(BF16) if USE_BF16 else consts["W1p"],
            "w2": w2,
            "t1": t1,
            "t2": t2,
            "fcw": fcw,
        })
    return nc, in_maps, consts


def execute(nc, in_maps):
    from concourse.bass_utils import run_bass_kernel_spmd
    return run_bass_kernel_spmd(nc, in_maps, core_ids=list(range(NCORES)))


def unshard(res, consts):
    y = np.zeros((N_NODES, 1), np.float32)
    nos = consts["node_of_slot"]
    fcb = consts["fcb"]
    for c in range(NCORES):
        nodes = nos[c * SPC:(c + 1) * SPC]
        occ = nodes >= 0
        vals = res.results[c]["y"].T.reshape(-1) + fcb
        y[nodes[occ], 0] = vals[occ]
    return y


def kernel(**inputs):
    nc, in_maps, consts = prepare(inputs)
    res = execute(nc, in_maps)
    return unshard(res, consts)


# revision 19
# speedup vs baseline: 1826.9474x; 1.0497x over previous
"""Distributed 2-layer GCN (BangaloreGCN) on 8 Trainium2 NeuronCores.

Strategy (node/graph-parallel, per spec sharding hint):
  * Nodes are packed into 8*49 destination tiles of 128 slots (LPT on
    in-degree so every tile's incoming-edge count fits a fixed chunk
    budget -> fully static SPMD program).
  * GCN algebra is refactored so message passing is a pure gather +
    segment-sum:  out = dinv * (A @ (dinv*h)) + dinv^2 * h, with the
    per-channel BN scale folded into W, biases folded into a post-add.
  * Per layer: each core computes its shard of the (dinv*h) table,
    AllGather replicates it to HBM on every core, then each core
    dma_gathers the source rows for its own edges and segment-sums them
    with one-hot selection matmuls into PSUM (128 dests x 64 feats).
  * int16 gather indices only span 32768 rows, so edges are split into
    a "low" pass (table rows [0, 32768)) and "high" pass (rows
    [NSLOT-32768, NSLOT)); edges in the overlap are assigned to balance
    per-tile chunk counts.
"""

import sys

sys.path.insert(0, "/opt/trn_rl_repo")

import heapq

import ml_dtypes
import numpy as np

BF16 = ml_dtypes.bfloat16

# ---- problem constants (hardcoded per contest contract) ----
N_NODES = 50000
IN_CH = 128
HID = 64
HID2 = 32
BN_EPS = 1e-5

NCORES = 8
P = 128
TILES = 49                 # dest tiles per core
SPC = TILES * P            # slots per core (6272)
NSLOT = NCORES * SPC       # 50176
NBINS = NCORES * TILES
LO_LIM = 32768             # low gather table covers rows [0, 32768)
HI_BASE = NSLOT - 32768    # high table covers [HI_BASE, NSLOT)
GT = 7                     # dest tiles per dma_gather call
NCALLS = TILES // GT
PAD_DEST = 200.0
TBW = 128                  # padded table row width (bf16 -> 256B elems)

USE_BF16 = True


# ----------------------------------------------------------------------
# host-side preparation
# ----------------------------------------------------------------------
def _pack_nodes(deg_in, n):
    order = np.argsort(-deg_in, kind="stable")
    heap = [(0, b) for b in range(NBINS)]
    heapq.heapify(heap)
    counts = np.zeros(NBINS, np.int32)
    binof = np.empty(n, np.int32)
    for v in order:
        load, b = heapq.heappop(heap)
        binof[v] = b
        counts[b] += 1
        if counts[b] < P:
            heapq.heappush(heap, (load + int(deg_in[v]), b))
    perm = np.argsort(binof, kind="stable")
    ptr = np.zeros(NBINS, np.int32)
    lanes = np.empty(n, np.int32)
    for v in perm:
        b = binof[v]
        lanes[v] = ptr[b]
        ptr[b] += 1
    return binof.astype(np.int64) * P + lanes


def _wrap_idx(arr):
    ni = arr.shape[0]
    blk = arr.reshape(ni // 16, 16).T.astype(np.int16)
    return np.tile(blk, (8, 1))


def host_prep(x, edge_index, W1, b1, W2, b2, fcW, fcb,
              g1, be1, rm1, rv1, g2, be2, rm2, rv2):
    n = x.shape[0]
    row = np.asarray(edge_index[0], np.int64)
    col = np.asarray(edge_index[1], np.int64)

    deg = np.bincount(col, minlength=n).astype(np.float32) + 1.0
    dinv = (1.0 / np.sqrt(deg)).astype(np.float32)
    deg_in = np.bincount(col, minlength=n)

    slot_of_node = _pack_nodes(deg_in, n)
    node_of_slot = np.full(NSLOT, -1, np.int64)
    node_of_slot[slot_of_node] = np.arange(n)

    src_slot = slot_of_node[row]
    dst_slot = slot_of_node[col]
    dbin = dst_slot // P
    dlane = dst_slot % P

    order = np.argsort(dbin, kind="stable")
    src_s = src_slot[order]
    dlane_s = dlane[order]
    dbin_s = dbin[order]
    starts = np.searchsorted(dbin_s, np.arange(NBINS))
    ends = np.searchsorted(dbin_s, np.arange(NBINS) + 1)

    nA_min = np.zeros(NBINS, np.int64)
    nB_min = np.zeros(NBINS, np.int64)
    tot = ends - starts
    for b in range(NBINS):
        s = src_s[starts[b]:ends[b]]
        nA_min[b] = int((s < HI_BASE).sum())
        nB_min[b] = int((s >= LO_LIM).sum())
    maxA, maxB, maxT = int(nA_min.max()), int(nB_min.max()), int(tot.max())
    best = None
    for ct in range(-(-maxT // P), -(-maxT // P) + 8):
        for ca in range(-(-maxA // P), ct + 1):
            cb = ct - ca
            if cb >= 0 and cb * P >= maxB:
                best = (ca, cb)
                break
        if best:
            break
    CA, CB = best
    capA, capB = CA * P, CB * P

    srcA = np.zeros((NBINS, capA), np.int64)
    destA = np.full((NBINS, capA), PAD_DEST, np.float32)
    srcB = np.zeros((NBINS, capB), np.int64)
    destB = np.full((NBINS, capB), PAD_DEST, np.float32)
    for b in range(NBINS):
        s = src_s[starts[b]:ends[b]]
        d = dlane_s[starts[b]:ends[b]]
        isB_must = s >= LO_LIM
        isA_must = s < HI_BASE
        mid_idx = np.where(~isB_must & ~isA_must)[0]
        room = capB - int(isB_must.sum())
        takeB = mid_idx[:room]
        selB = np.concatenate([np.where(isB_must)[0], takeB])
        selA = np.concatenate([np.where(isA_must)[0], mid_idx[room:]])
        assert len(selB) <= capB and len(selA) <= capA
        srcB[b, :len(selB)] = s[selB] - HI_BASE
        destB[b, :len(selB)] = d[selB]
        srcA[b, :len(selA)] = s[selA]
        destA[b, :len(selA)] = d[selA]

    S1c = (g1 / np.sqrt(rv1 + BN_EPS)).astype(np.float32)
    T1 = ((b1 - rm1) * S1c + be1).astype(np.float32)
    S2c = (g2 / np.sqrt(rv2 + BN_EPS)).astype(np.float32)
    T2 = ((b2 - rm2) * S2c + be2).astype(np.float32)
    W1p = (W1 * S1c[None, :]).astype(np.float32)
    W2p = (W2 * S2c[None, :]).astype(np.float32)

    NCH = CA + CB
    cores = []
    for c in range(NCORES):
        tsl = slice(c * TILES, (c + 1) * TILES)
        sA = srcA[tsl].reshape(-1)
        sB = srcB[tsl].reshape(-1)
        idxA_img = np.hstack(
            [_wrap_idx(sA[g * GT * capA:(g + 1) * GT * capA]) for g in range(NCALLS)])
        idxB_img = np.hstack(
            [_wrap_idx(sB[g * GT * capB:(g + 1) * GT * capB]) for g in range(NCALLS)])
        dst_img = np.zeros((P, TILES * NCH), np.float32)
        for tl in range(TILES):
            b = c * TILES + tl
            dst_img[:, tl * NCH:tl * NCH + CA] = destA[b].reshape(CA, P).T
            dst_img[:, tl * NCH + CA:(tl + 1) * NCH] = destB[b].reshape(CB, P).T
        nodes = node_of_slot[c * SPC:(c + 1) * SPC]
        occ = nodes >= 0
        xs = np.zeros((SPC, IN_CH), np.float32)
        xs[occ] = x[nodes[occ]] * dinv[nodes[occ], None]
        dv = np.zeros(SPC, np.float32)
        dv[occ] = dinv[nodes[occ]]
        cores.append(dict(
            idxA=idxA_img, idxB=idxB_img,
            dest=dst_img.astype(BF16) if USE_BF16 else dst_img,
            xT=np.ascontiguousarray(xs.T),
            dinv=np.ascontiguousarray(dv.reshape(TILES, P).T),
        ))

    consts = dict(W1p=W1p, W2p=W2p, T1=T1, T2=T2,
                  fcW=np.asarray(fcW, np.float32), fcb=float(np.asarray(fcb).reshape(-1)[0]),
                  CA=CA, CB=CB, node_of_slot=node_of_slot)
    return cores, consts


# ----------------------------------------------------------------------
# device program
# ----------------------------------------------------------------------
def _dma_gather_raw(gp, bassmod, out_ap, in_ap, idxs_ap, num_idxs, elem_size,
                    elem_step, single_packet=True, queue_num=0):
    """bass.dma_gather with elem_size_bytes below 256B allowed (stride must
    still be a multiple of 256B). Verified on HW (see work/smoke4.py)."""
    import concourse.mybir as mybir
    from concourse import ap_utils
    from concourse.bass import MemorySpace, exact_div, round_up_to_multiple

    assert idxs_ap.dtype == mybir.dt.int16
    assert in_ap.dtype == out_ap.dtype
    assert in_ap.space == MemorySpace.DRAM
    assert idxs_ap.space == MemorySpace.SBUF and out_ap.space == MemorySpace.SBUF
    assert ap_utils.ap_is_contiguous(out_ap.ap[1:])
    assert ap_utils.ap_is_contiguous(idxs_ap.ap[1:])
    assert in_ap.ap[-1][1] == out_ap.ap[-1][1] == elem_size
    assert out_ap.ap[0][1] * out_ap.ap[1][1] == round_up_to_multiple(num_idxs, 128)
    assert in_ap.ap[0][0] == elem_step
    stride_bytes_256 = exact_div(elem_step * mybir.dt.size(in_ap.dtype), 256)
    assert stride_bytes_256 < 256
    return gp.add_instruction(
        mybir.InstDMAGatherAnt(
            name=bassmod.get_next_instruction_name(),
            ins=[*gp.lower_ap_dma(in_ap, for_custom_bir_dma=True),
                 gp.lower_ap(idxs_ap),
                 gp.lower_val_access(gp.to_reg(num_idxs))],
            outs=[gp.lower_ap(out_ap)],
            transpose=False,
            num_idxs=num_idxs,
            elem_size=elem_size,
            stride_bytes_256=stride_bytes_256,
            gen_mode=0,
            single_packet=single_packet,
            queue_num=queue_num,
            sbuf_tokens_per_rank=0,
            sbuf_free_dim_per_rank=0,
            sbuf_free_dim_pad_per_rank=0,
            sbuf_byte_offset=0,
        ))


def build_bass(CA, CB):
    import concourse.bacc as bacc
    import concourse.bass as bassm
    import concourse.mybir as mybir
    import concourse.tile as tile
    from concourse.library_config import mlp
    from concourse.masks import make_identity

    f32 = mybir.dt.float32
    bf = mybir.dt.bfloat16 if USE_BF16 else f32
    i16 = mybir.dt.int16
    tbw = TBW if USE_BF16 else HID
    NCH = CA + CB
    capA, capB = CA * P, CB * P
    wA = GT * capA // 16
    wB = GT * capB // 16

    nc = bacc.Bacc("TRN2", target_bir_lowering=False)
    xT_d = nc.dram_tensor("xT", [P, SPC], bf, kind="ExternalInput")
    idxA_d = nc.dram_tensor("idxA", [P, TILES * capA // 16], i16, kind="ExternalInput")
    idxB_d = nc.dram_tensor("idxB", [P, TILES * capB // 16], i16, kind="ExternalInput")
    dest_d = nc.dram_tensor("dest", [P, TILES * NCH], bf, kind="ExternalInput")
    dinv_d = nc.dram_tensor("dinv", [P, TILES], f32, kind="ExternalInput")
    w1_d = nc.dram_tensor("w1", [IN_CH, HID], bf, kind="ExternalInput")
    w2_d = nc.dram_tensor("w2", [HID, HID2], f32, kind="ExternalInput")
    t1_d = nc.dram_tensor("t1", [P, HID], f32, kind="ExternalInput")
    t2_d = nc.dram_tensor("t2", [P, HID2], f32, kind="ExternalInput")
    fcw_d = nc.dram_tensor("fcw", [P, HID2], f32, kind="ExternalInput")
    y_d = nc.dram_tensor("y", [P, TILES], f32, kind="ExternalOutput")

    with tile.TileContext(nc) as tc:
        with (
            tc.tile_pool(name="const", bufs=1) as cpool,
            tc.tile_pool(name="upart", bufs=1) as upool,
            tc.tile_pool(name="ga", bufs=3) as gapool,
            tc.tile_pool(name="gb", bufs=2) as gbpool,
            tc.tile_pool(name="sel", bufs=20) as selpool,
            tc.tile_pool(name="work", bufs=4) as wpool,
            tc.tile_pool(name="pmm", bufs=2, space="PSUM") as pmm,
            tc.tile_pool(name="pacc", bufs=3, space="PSUM") as pacc,
            tc.tile_pool(name="ptr", bufs=1, space="PSUM") as ptr,
            tc.tile_pool(name="p3", bufs=2, space="PSUM") as p3pool,
            tc.tile_pool(name="dram", bufs=1, space="DRAM") as dpool,
        ):
            nc.gpsimd.load_library(mlp)

            # ---- constants ----
            idxA_t = cpool.tile([P, TILES * capA // 16], i16)
            nc.sync.dma_start(out=idxA_t[:], in_=idxA_d[:])
            idxB_t = cpool.tile([P, TILES * capB // 16], i16)
            nc.sync.dma_start(out=idxB_t[:], in_=idxB_d[:])
            dest_t = cpool.tile([P, TILES * NCH], bf)
            nc.sync.dma_start(out=dest_t[:], in_=dest_d[:])
            dinv_t = cpool.tile([P, TILES], f32)
            nc.sync.dma_start(out=dinv_t[:], in_=dinv_d[:])
            w1_t = cpool.tile([IN_CH, HID], bf)
            nc.sync.dma_start(out=w1_t[:], in_=w1_d[:])
            w2_t = cpool.tile([HID, HID2], f32)
            nc.sync.dma_start(out=w2_t[:], in_=w2_d[:])
            t1_t = cpool.tile([P, HID], f32)
            nc.sync.dma_start(out=t1_t[:], in_=t1_d[:])
            t2_t = cpool.tile([P, HID2], f32)
            nc.sync.dma_start(out=t2_t[:], in_=t2_d[:])
            fcw_t = cpool.tile([P, HID2], f32)
            nc.sync.dma_start(out=fcw_t[:], in_=fcw_d[:])

            ident = cpool.tile([P, P], f32)
            make_identity(nc, ident[:])
            ones_row = cpool.tile([1, P], f32)
            nc.gpsimd.memset(ones_row[:], 1.0)
            iota_i = cpool.tile([P, NCH * P], mybir.dt.int32)
            nc.gpsimd.iota(iota_i[:], pattern=[[0, NCH], [1, P]], base=0,
                           channel_multiplier=0)
            iota_b = cpool.tile([P, NCH * P], bf)
            nc.vector.tensor_copy(out=iota_b[:], in_=iota_i[:])

            u1_t = upool.tile([P, TILES * HID], f32, tag="u1")
            s2_t = upool.tile([P, TILES * HID], f32, tag="s2")
            out_t = upool.tile([P, TILES], f32, tag="out")

            ag1_in = dpool.tile([SPC, tbw], bf)
            s1_tab = dpool.tile([NSLOT, tbw], bf, addr_space="Shared")
            ag2_in = dpool.tile([SPC, tbw], bf)
            s2_tab = dpool.tile([NSLOT, tbw], bf, addr_space="Shared")

            # ---- L1 dense: u = (x*dinv) @ W1' ----
            xfull = cpool.tile([P, SPC], bf)
            nc.sync.dma_start(out=xfull[:], in_=xT_d[:])
            for t in range(TILES):
                pm = pmm.tile([P, HID], f32, space="PSUM", tag="pm")
                nc.tensor.matmul(out=pm[:], lhsT=xfull[:, t * P:(t + 1) * P],
                                 rhs=w1_t[:], start=True, stop=True)
                nc.scalar.activation(out=u1_t[:, t * HID:(t + 1) * HID], in_=pm[:],
                                     func=mybir.ActivationFunctionType.Copy)

            nc.gpsimd.dma_start(
                out=ag1_in[:].rearrange("(t p) w -> p t w", p=P)[:, :, 0:HID],
                in_=u1_t[:].rearrange("p (t f) -> p t f", f=HID),
            )
            nc.gpsimd.collective_compute(
                "AllGather", mybir.AluOpType.bypass,
                replica_groups=[list(range(NCORES))],
                ins=[ag1_in[:]], outs=[s1_tab[:]],
            )

            def tab_ap(tab, lo, cnt):
                return bassm.AP(tensor=tab[:].tensor, offset=lo * tbw,
                                ap=[[tbw, cnt], [1, HID]])

            def scatter_tiles(tab, u_tab, post):
                for g in range(NCALLS):
                    ga = gapool.tile([P, GT * CA, HID], bf, tag="ga")
                    _dma_gather_raw(
                        nc.gpsimd, nc, ga[:], tab_ap(tab, 0, LO_LIM),
                        idxA_t[:, g * wA:(g + 1) * wA], GT * capA, HID, tbw,
                        single_packet=False)
                    gb = gbpool.tile([P, GT * CB, HID], bf, tag="gb")
                    _dma_gather_raw(
                        nc.gpsimd, nc, gb[:], tab_ap(tab, HI_BASE, LO_LIM),
                        idxB_t[:, g * wB:(g + 1) * wB], GT * capB, HID, tbw,
                        single_packet=False)
                    for k in range(GT):
                        t = g * GT + k
                        sel = selpool.tile([P, NCH, P], bf, tag="sel")
                        nc.vector.tensor_tensor(
                            out=sel[:],
                            in0=dest_t[:, t * NCH:(t + 1) * NCH, None]
                                .to_broadcast([P, NCH, P]),
                            in1=iota_b[:].rearrange("p (c q) -> p c q", q=P),
                            op=mybir.AluOpType.is_equal,
                        )
                        acc = pacc.tile([P, HID], f32, space="PSUM", tag="acc")
                        for cc in range(NCH):
                            rhs = (ga[:, k * CA + cc, :] if cc < CA
                                   else gb[:, k * CB + cc - CA, :])
                            nc.tensor.matmul(out=acc[:], lhsT=sel[:, cc, :], rhs=rhs,
                                             start=(cc == 0), stop=False)
                        nc.tensor.matmul(out=acc[:], lhsT=ident[:],
                                         rhs=u_tab[:, t * HID:(t + 1) * HID],
                                         start=False, stop=True)
                        post(t, acc)

            # ---- L1 scatter + post: s2 = dinv * relu(dinv*(acc+u) + T1) ----
            def post1(t, acc):
                tmp2 = wpool.tile([P, HID], f32, tag="tmp2")
                nc.scalar.activation(out=tmp2[:], in_=acc[:],
                                     func=mybir.ActivationFunctionType.Copy,
                                     scale=dinv_t[:, t:t + 1])
                h1 = wpool.tile([P, HID], f32, tag="h1")
                nc.vector.tensor_tensor(out=h1[:], in0=tmp2[:], in1=t1_t[:],
                                        op=mybir.AluOpType.add)
                h1r = wpool.tile([P, HID], f32, tag="h1r")
                nc.scalar.activation(out=h1r[:], in_=h1[:],
                                     func=mybir.ActivationFunctionType.Relu)
                nc.scalar.activation(out=s2_t[:, t * HID:(t + 1) * HID],
                                     in_=h1r[:],
                                     func=mybir.ActivationFunctionType.Copy,
                                     scale=dinv_t[:, t:t + 1])

            scatter_tiles(s1_tab, u1_t, post1)

            nc.gpsimd.dma_start(
                out=ag2_in[:].rearrange("(t p) w -> p t w", p=P)[:, :, 0:HID],
                in_=s2_t[:].rearrange("p (t f) -> p t f", f=HID),
            )
            nc.gpsimd.collective_compute(
                "AllGather", mybir.AluOpType.bypass,
                replica_groups=[list(range(NCORES))],
                ins=[ag2_in[:]], outs=[s2_tab[:]],
            )

            # ---- L2 scatter + post ----
            def post2(t, acc):
                agg = wpool.tile([P, HID], f32, tag="agg")
                nc.scalar.activation(out=agg[:], in_=acc[:],
                                     func=mybir.ActivationFunctionType.Copy,
                                     scale=dinv_t[:, t:t + 1])
                trp = ptr.tile([HID, P], f32, space="PSUM", tag="trp")
                nc.tensor.transpose(out=trp[:], in_=agg[:], identity=ident[:])
                aggT = wpool.tile([HID, P], f32, tag="aggT")
                nc.scalar.activation(out=aggT[:], in_=trp[:],
                                     func=mybir.ActivationFunctionType.Copy)
                p3 = p3pool.tile([P, HID2], f32, space="PSUM", tag="p3")
                nc.tensor.matmul(out=p3[:], lhsT=aggT[:], rhs=w2_t[:],
                                 start=True, stop=False)
                nc.tensor.matmul(out=p3[:], lhsT=ones_row[:], rhs=t2_t[0:1, :],
                                 start=False, stop=True)
                h2 = wpool.tile([P, HID2], f32, tag="h2")
                nc.scalar.activation(out=h2[:], in_=p3[:],
                                     func=mybir.ActivationFunctionType.Relu)
                prod = wpool.tile([P, HID2], f32, tag="prod")
                nc.vector.tensor_tensor(out=prod[:], in0=h2[:], in1=fcw_t[:],
                                        op=mybir.AluOpType.mult)
                nc.vector.reduce_sum(out=out_t[:, t:t + 1], in_=prod[:],
                                     axis=mybir.AxisListType.X)

            scatter_tiles(s2_tab, s2_t, post2)

            nc.sync.dma_start(out=y_d[:], in_=out_t[:])

    nc.compile()
    return nc


# ----------------------------------------------------------------------
# entry point
# ----------------------------------------------------------------------
def prepare(inputs):
    inputs = {k: np.asarray(v) for k, v in inputs.items()}
    cores, consts = host_prep(**inputs)
    nc = build_bass(consts["CA"], consts["CB"])

    cast = BF16 if USE_BF16 else np.float32
    w2 = consts["W2p"].astype(np.float32)
    t1 = np.tile(consts["T1"][None, :], (P, 1)).astype(np.float32)
    t2 = np.tile(consts["T2"][None, :], (P, 1)).astype(np.float32)
    fcw = np.tile(consts["fcW"].reshape(1, -1), (P, 1)).astype(np.float32)

    in_maps = []
    for c in range(NCORES):
        in_maps.append({
            "xT": cores[c]["xT"].astype(BF16) if USE_BF16 else cores[c]["xT"],
            "idxA": cores[c]["idxA"],
            "idxB": cores[c]["idxB"],
            "dest": cores[c]["dest"].astype(cast),
            "dinv": cores[c]["dinv"],
            "w1": consts["W1p"].astype(BF16) if USE_BF16 else consts["W1p"],
            "w2": w2,
            "t1": t1,
            "t2": t2,
            "fcw": fcw,
        })
    return nc, in_maps, consts


def execute(nc, in_maps):
    from concourse.bass_utils import run_bass_kernel_spmd
    return run_bass_kernel_spmd(nc, in_maps, core_ids=list(range(NCORES)))


def unshard(res, consts):
    y = np.zeros((N_NODES, 1), np.float32)
    nos = consts["node_of_slot"]
    fcb = consts["fcb"]
    for c in range(NCORES):
        nodes = nos[c * SPC:(c + 1) * SPC]
        occ = nodes >= 0
        vals = res.results[c]["y"].T.reshape(-1) + fcb
        y[nodes[occ], 0] = vals[occ]
    return y


def kernel(**inputs):
    nc, in_maps, consts = prepare(inputs)
    res = execute(nc, in_maps)
    return unshard(res, consts)
